# revision 22
# baseline (speedup 1.0000x reference)
"""Pairwise cosine-distance matrix kernel for Trainium2 (Bass/Tile, 8 cores).

Problem: mapping [8192, 512] fp32 -> out[i,j] = 1 - <x_i, x_j> / (|x_i||x_j|),
full [8192, 8192] fp32 output.

Strategy (SPMD over 8 NeuronCores, symmetric-triangle partitioning):
  - The output is symmetric, so only the 136 distinct [512, 512] blocks of
    the 16x16 block grid need device compute. Circulant assignment: row-block
    r computes blocks (r, r+d mod 16) for d = 0..7, and the 8 "bridge"
    blocks (c, c+8) go one per core. Core c owns row-blocks c and c+8 ->
    exactly 17 blocks per core, with a uniform structure (weight A x 9
    column tiles + weight B x 8 column tiles) so a single NEFF serves all
    cores SPMD.
  - The host rotates the transposed matrix's columns by 512*c per core so
    that each core's weight/moving slices sit at identical compile-time
    offsets. Host input is fp16 (halves input DMA; the 2e-2 rel-err budget
    dwarfs fp16 quantization).
  - On device: per 512-column tile, compute column norms (ACT square ->
    DVE elementwise-sum of the 4 k-chunks -> ones-matmul partition reduce
    -> ACT rsqrt -> K=1 broadcast matmul) and scale the tile in place
    (fp16). The gram blocks run as fp16 matmuls (1 PE cycle/row) with fp32
    PSUM accumulation and a fused (1 - x) epilogue split across ACT/DVE
    into fp16 staging tiles, then DMA out.
  - Host upcasts to fp32, places each block, and mirrors its transpose to
    the symmetric position.
"""

import json
import os
import sys
import types

import numpy as np

N = 8192
D = 512
N_CORES = 8
NB = 16                 # 512-wide row/col blocks
BS = N // NB            # 512
KC = D // 128           # 4 k-chunks of 128
MT = BS // 128          # 4 row-chunks of 128 per 512-row part

# tiles normalized before gram group g runs (norm group g gates gram group g)
NORM_GROUPS = [[0, 1, 2], [3, 4, 5], [6, 7, 8], [9, 10], [11, 12, 13], [14, 15]]
# (weight_tile, moving_tiles, out_name, out_col_offset)
GRAM_GROUPS = [
    (0, [0, 1, 2], "outA", 0),
    (0, [3, 4, 5], "outA", 3 * BS),
    (0, [6, 7, 8], "outA", 6 * BS),
    (8, [8, 9, 10], "outB", 0),
    (8, [11, 12, 13], "outB", 3 * BS),
    (8, [14, 15], "outB", 6 * BS),
]

LAST_EXEC_NS = None  # max-across-traced-cores HW time of the last profiled run

_cached = {}


def _install_ntff_hook():
    """bass_utils' trace path imports antenv.axon_hooks, which this image
    lacks; recreate it and register the ctypes NTFF hook (same thing the
    boot script would have done)."""
    if "antenv.axon_hooks" in sys.modules:
        return
    mod = types.ModuleType("antenv.axon_hooks")
    holder = [None]
    mod.set_axon_ntff_profile_hook = lambda h: holder.__setitem__(0, h)
    mod.get_axon_ntff_profile_hook = lambda: holder[0]
    sys.modules["antenv.axon_hooks"] = mod
    import antenv
    antenv.axon_hooks = mod
    try:
        from trn_agent_boot.trn_boot import _ntff_profile_via_ctypes
        mod.set_axon_ntff_profile_hook(
            _ntff_profile_via_ctypes("/opt/axon/libaxon_pjrt.so")
        )
    except Exception:
        pass


def _split_multiwait_bir(bir_json: bytes) -> bytes:
    """This container's walrus rejects instructions with >1 semaphore wait
    ("Too many sync wait commands"). Hoist extra waits onto standalone
    wait-only EventSemaphore instructions placed just before, on the same
    engine — identical stall semantics."""
    m = json.loads(bir_json)
    for f in m["functions"]:
        for bb in f.get("blocks", f.get("basicblocks", [])):
            new_insts = []
            for inst in bb["instructions"]:
                si = inst.get("sync_info")
                waits = si.get("on_wait") if si else None
                if waits and len(waits) > 1:
                    for j, w in enumerate(waits[:-1]):
                        new_insts.append({
                            "debug": inst.get("debug"),
                            "engine": inst["engine"],
                            "ins": [],
                            "name": f"{inst['name']}-hw{j}",
                            "opcode": "EventSemaphore",
                            "outs": [],
                            "sync_info": {"on_update": [], "on_wait": [w]},
                        })
                    si["on_wait"] = [waits[-1]]
                new_insts.append(inst)
            bb["instructions"] = new_insts
    return json.dumps(m).encode()


def _apply_patches():
    if _cached.get("patched"):
        return
    _cached["patched"] = True
    import concourse.bass2jax as bass2jax
    import concourse.bass_utils as bass_utils

    orig_compile = bass2jax.compile_bir_kernel

    def patched_compile(bir_json, tmpdir, neff_name="file.neff"):
        return orig_compile(_split_multiwait_bir(bir_json), tmpdir,
                            neff_name=neff_name)

    bass2jax.compile_bir_kernel = patched_compile
    # No S3 in this container; the trace path uploads artifacts for links only.
    bass_utils.upload_artifacts = lambda tmpdir: "local://" + tmpdir


def _build():
    key = "nc"
    if key in _cached:
        return _cached[key]
    _apply_patches()
    import concourse.bass as bass
    import concourse.tile as tile
    from concourse import mybir

    f32 = mybir.dt.float32
    f16 = mybir.dt.float16
    Act = mybir.ActivationFunctionType
    Alu = mybir.AluOpType

    nc = bass.Bass(trn_type="TRN2", target_bir_lowering=False, debug=False)
    xt_d = nc.dram_tensor("xt", [D, N], f16, kind="ExternalInput").ap()
    outA_d = nc.dram_tensor("outA", [BS, 9 * BS], f16, kind="ExternalOutput").ap()
    outB_d = nc.dram_tensor("outB", [BS, 8 * BS], f16, kind="ExternalOutput").ap()

    with tile.TileContext(nc) as tc:
        with (
            tc.tile_pool(name="xt", bufs=1) as xt_pool,
            tc.tile_pool(name="sq", bufs=4) as sq_pool,
            tc.tile_pool(name="tmp", bufs=10) as tmp_pool,
            tc.tile_pool(name="nrm", bufs=9) as nrm_pool,
            tc.tile_pool(name="rows", bufs=4) as row_pool,
            tc.tile_pool(name="bc16", bufs=5) as bc_pool,
            tc.tile_pool(name="consts", bufs=1) as const_pool,
            tc.tile_pool(name="stage", bufs=3) as stage_pool,
            tc.tile_pool(name="ps_nb", bufs=2, space=bass.MemorySpace.PSUM) as ps_nb,
            tc.tile_pool(name="ps_g", bufs=2, space=bass.MemorySpace.PSUM) as ps_g,
        ):
            ones_col = const_pool.tile([128, 1], f16, name="ones_col")
            nc.vector.memset(ones_col[:], 1.0)
            one_bias = const_pool.tile([128, 1], f32, name="one_bias")
            nc.vector.memset(one_bias[:], 1.0)

            # xt is tile-major: 512-col tile t occupies [2048t, 2048(t+1)),
            # with k-chunk k at +512k. Every op below then works on plain
            # contiguous 2-dim slices (exact ranges for dep tracking), and
            # the per-tile square/scale run as ONE wide fp16 2x-mode DVE op.
            TW = KC * BS                      # 2048 cols per tile
            xt = xt_pool.tile([128, NB * TW], f16, name="xt")

            def xtile(t):
                return xt[:, TW * t:TW * (t + 1)]

            def xmov(k, t):
                return xt[:, TW * t + BS * k:TW * t + BS * (k + 1)]

            def xw(k, wt, mt):
                base = TW * wt + BS * k + 128 * mt
                return xt[:, base:base + 128]

            # tile-major input DMA, norm-group order
            for tiles in NORM_GROUPS:
                for t in tiles:
                    for k in range(KC):
                        nc.sync.dma_start(
                            out=xmov(k, t),
                            in_=xt_d[k * 128:(k + 1) * 128,
                                     t * BS:(t + 1) * BS])

            ones_row = const_pool.tile([1, 128], f16, name="ones_row")
            nc.vector.memset(ones_row[:], 1.0)

            def normalize(tiles, gid):
                """Column-normalize the 512-wide tiles in place (fp16).
                Per tile: one fused DVE square over the contiguous
                [128, 2048] tile -> 2-level DVE adds -> ones-matmul
                partition reduce -> rsqrt as exp(-0.5*ln(x)) on ACT (the
                [1,512] row lives on one partition, where DVE reciprocal
                costs ~6.3 ns/elem; ACT Rsqrt is blocked for accuracy) ->
                K=1 broadcast matmul -> ACT f16 copy -> one fused in-place
                fp16 2x-mode DVE scale. No DMA in the chain: measured,
                DRAM-bounce variants convoy the DMA queues and lose."""
                for i, t in enumerate(tiles):
                    sq = sq_pool.tile([128, TW], f16, tag="sq", name=f"sq{t}")
                    nc.vector.tensor_mul(sq[:], xtile(t), xtile(t))
                    a01 = tmp_pool.tile([128, 2 * BS], f16, tag="tmp",
                                        name=f"a01_{t}")
                    nc.vector.tensor_add(a01[:], sq[:, 0:2 * BS],
                                         sq[:, 2 * BS:4 * BS])
                    ssum = tmp_pool.tile([128, BS], f16, tag="tmp",
                                         name=f"ssum_{t}")
                    nc.vector.tensor_add(ssum[:], a01[:, 0:BS], a01[:, BS:])
                    n2 = ps_nb.tile([1, BS], f32, tag="nb", name=f"n2_{t}")
                    nc.tensor.matmul(n2[:], ones_col[:], ssum[:],
                                     start=True, stop=True)
                    lnx = tmp_pool.tile([1, BS], f32, tag="tmp",
                                        name=f"ln_{t}")
                    nc.scalar.activation(lnx[:], n2[:], Act.Ln)
                    rn = row_pool.tile([1, BS], f16, tag="rn",
                                       name=f"rn_{gid}_{i}")
                    nc.scalar.activation(rn[:], lnx[:], Act.Exp, scale=-0.5)
                    bc = ps_nb.tile([128, BS], f32, tag="nb", name=f"bc_{t}")
                    nc.tensor.matmul(bc[:], ones_row[:], rn[:],
                                     start=True, stop=True)
                    bc16 = bc_pool.tile([128, BS], f16, tag="bc",
                                        name=f"bc16_{t}")
                    if i % 2 == 0:
                        nc.scalar.copy(bc16[:], bc[:])
                    else:
                        nc.vector.tensor_copy(bc16[:], bc[:])
                    nc.vector.tensor_mul(
                        xtile(t).rearrange("p (a c) -> p a c", a=KC),
                        xtile(t).rearrange("p (a c) -> p a c", a=KC),
                        bc16[:].unsqueeze(1).broadcast_to((128, KC, BS)))

            normalize(NORM_GROUPS[0], 0)
            normalize(NORM_GROUPS[1], 1)

            for gi, (wt, tiles, out_name, off) in enumerate(GRAM_GROUPS):
                out_d = outA_d if out_name == "outA" else outB_d
                nt = len(tiles)
                for mt in range(MT):
                    # prefetch norm group gi+2 under this group's gram
                    # stream; high_priority floats it as early as deps allow
                    if mt == 1 and gi + 2 < len(NORM_GROUPS):
                        with tc.high_priority():
                            normalize(NORM_GROUPS[gi + 2], gi + 2)
                    # one wide psum tile per mt (nt<=3 banks): the
                    # epilogue then reads all nt gram tiles in ONE pass,
                    # amortizing the per-op constant (~330 ACT cycles)
                    psum = ps_g.tile([128, nt * BS], f32, tag="pg",
                                     name=f"pg_{gi}_{mt}")
                    for k in range(KC):
                        for j, t in enumerate(tiles):
                            nc.tensor.matmul(psum[:, j * BS:(j + 1) * BS],
                                             xw(k, wt, mt), xmov(k, t),
                                             start=(k == 0), stop=(k == KC - 1))
                    stage = stage_pool.tile([128, nt * BS], f16, tag="st",
                                            name=f"st_{gi}_{mt}")
                    nc.scalar.activation(stage[:], psum[:], Act.Identity,
                                         bias=one_bias[:], scale=-1.0)
                    nc.sync.dma_start(
                        out=out_d[mt * 128:(mt + 1) * 128, off:off + nt * BS],
                        in_=stage[:])

    _cached[key] = nc
    return nc


def kernel(mapping: np.ndarray) -> np.ndarray:
    from concourse.bass_utils import run_bass_kernel_spmd

    mapping = np.ascontiguousarray(mapping, dtype=np.float32)
    assert mapping.shape == (N, D)
    xt16 = np.ascontiguousarray(mapping.T.astype(np.float16))  # [512, 8192]
    in_maps = []
    for c in range(N_CORES):
        in_maps.append({"xt": np.ascontiguousarray(
            np.roll(xt16, -BS * c, axis=1))})

    nc = _build()

    trace = bool(int(os.environ.get("BASSKNN_TRACE", "0")))
    if trace:
        _install_ntff_hook()
    res = run_bass_kernel_spmd(nc, in_maps, list(range(N_CORES)), trace=trace)
    global LAST_EXEC_NS
    if trace:
        LAST_EXEC_NS = res.exec_time_ns

    full = np.empty((N, N), np.float32)
    for c in range(N_CORES):
        A = np.asarray(res.results[c]["outA"]).astype(np.float32)
        B = np.asarray(res.results[c]["outB"]).astype(np.float32)
        for t in range(9):
            j = (c + t) % NB
            blk = A[:, t * BS:(t + 1) * BS]
            full[c * BS:(c + 1) * BS, j * BS:(j + 1) * BS] = blk
            if t:
                full[j * BS:(j + 1) * BS, c * BS:(c + 1) * BS] = blk.T
        i2 = c + 8
        for e in range(8):
            j = (i2 + e) % NB
            blk = B[:, e * BS:(e + 1) * BS]
            full[i2 * BS:(i2 + 1) * BS, j * BS:(j + 1) * BS] = blk
            if e:
                full[j * BS:(j + 1) * BS, i2 * BS:(i2 + 1) * BS] = blk.T
    return full


# revision 23
# speedup vs baseline: 1.0141x; 1.0141x over previous
"""Pairwise cosine-distance matrix kernel for Trainium2 (Bass/Tile, 8 cores).

Problem: mapping [8192, 512] fp32 -> out[i,j] = 1 - <x_i, x_j> / (|x_i||x_j|),
full [8192, 8192] fp32 output.

Strategy (SPMD over 8 NeuronCores, symmetric-triangle partitioning):
  - The output is symmetric, so only the 136 distinct [512, 512] blocks of
    the 16x16 block grid need device compute. Circulant assignment: row-block
    r computes blocks (r, r+d mod 16) for d = 0..7, and the 8 "bridge"
    blocks (c, c+8) go one per core. Core c owns row-blocks c and c+8 ->
    exactly 17 blocks per core, with a uniform structure (weight A x 9
    column tiles + weight B x 8 column tiles) so a single NEFF serves all
    cores SPMD.
  - The host rotates the transposed matrix's columns by 512*c per core so
    that each core's weight/moving slices sit at identical compile-time
    offsets. Host input is fp16 (halves input DMA; the 2e-2 rel-err budget
    dwarfs fp16 quantization).
  - On device: per 512-column tile, compute column norms (ACT square ->
    DVE elementwise-sum of the 4 k-chunks -> ones-matmul partition reduce
    -> ACT rsqrt -> K=1 broadcast matmul) and scale the tile in place
    (fp16). The gram blocks run as fp16 matmuls (1 PE cycle/row) with fp32
    PSUM accumulation and a fused (1 - x) epilogue split across ACT/DVE
    into fp16 staging tiles, then DMA out.
  - Host upcasts to fp32, places each block, and mirrors its transpose to
    the symmetric position.
"""

import json
import os
import sys
import types

import numpy as np

N = 8192
D = 512
N_CORES = 8
NB = 16                 # 512-wide row/col blocks
BS = N // NB            # 512
KC = D // 128           # 4 k-chunks of 128
MT = BS // 128          # 4 row-chunks of 128 per 512-row part

# tiles normalized before gram group g runs (norm group g gates gram group g)
NORM_GROUPS = [[0, 1, 2], [3, 4, 5], [6, 7, 8], [9, 10], [11, 12, 13], [14, 15]]
# (weight_tile, moving_tiles, out_name, out_col_offset)
GRAM_GROUPS = [
    (0, [0, 1, 2], "outA", 0),
    (0, [3, 4, 5], "outA", 3 * BS),
    (0, [6, 7, 8], "outA", 6 * BS),
    (8, [8, 9, 10], "outB", 0),
    (8, [11, 12, 13], "outB", 3 * BS),
    (8, [14, 15], "outB", 6 * BS),
]

LAST_EXEC_NS = None  # max-across-traced-cores HW time of the last profiled run

_cached = {}


def _install_ntff_hook():
    """bass_utils' trace path imports antenv.axon_hooks, which this image
    lacks; recreate it and register the ctypes NTFF hook (same thing the
    boot script would have done)."""
    if "antenv.axon_hooks" in sys.modules:
        return
    mod = types.ModuleType("antenv.axon_hooks")
    holder = [None]
    mod.set_axon_ntff_profile_hook = lambda h: holder.__setitem__(0, h)
    mod.get_axon_ntff_profile_hook = lambda: holder[0]
    sys.modules["antenv.axon_hooks"] = mod
    import antenv
    antenv.axon_hooks = mod
    try:
        from trn_agent_boot.trn_boot import _ntff_profile_via_ctypes
        mod.set_axon_ntff_profile_hook(
            _ntff_profile_via_ctypes("/opt/axon/libaxon_pjrt.so")
        )
    except Exception:
        pass


def _split_multiwait_bir(bir_json: bytes) -> bytes:
    """This container's walrus rejects instructions with >1 semaphore wait
    ("Too many sync wait commands"). Hoist extra waits onto standalone
    wait-only EventSemaphore instructions placed just before, on the same
    engine — identical stall semantics."""
    m = json.loads(bir_json)
    for f in m["functions"]:
        for bb in f.get("blocks", f.get("basicblocks", [])):
            new_insts = []
            for inst in bb["instructions"]:
                si = inst.get("sync_info")
                waits = si.get("on_wait") if si else None
                if waits and len(waits) > 1:
                    for j, w in enumerate(waits[:-1]):
                        new_insts.append({
                            "debug": inst.get("debug"),
                            "engine": inst["engine"],
                            "ins": [],
                            "name": f"{inst['name']}-hw{j}",
                            "opcode": "EventSemaphore",
                            "outs": [],
                            "sync_info": {"on_update": [], "on_wait": [w]},
                        })
                    si["on_wait"] = [waits[-1]]
                new_insts.append(inst)
            bb["instructions"] = new_insts
    return json.dumps(m).encode()


def _apply_patches():
    if _cached.get("patched"):
        return
    _cached["patched"] = True
    import concourse.bass2jax as bass2jax
    import concourse.bass_utils as bass_utils

    orig_compile = bass2jax.compile_bir_kernel

    def patched_compile(bir_json, tmpdir, neff_name="file.neff"):
        return orig_compile(_split_multiwait_bir(bir_json), tmpdir,
                            neff_name=neff_name)

    bass2jax.compile_bir_kernel = patched_compile
    # No S3 in this container; the trace path uploads artifacts for links only.
    bass_utils.upload_artifacts = lambda tmpdir: "local://" + tmpdir


def _build():
    key = "nc"
    if key in _cached:
        return _cached[key]
    _apply_patches()
    import concourse.bass as bass
    import concourse.tile as tile
    from concourse import mybir

    f32 = mybir.dt.float32
    f16 = mybir.dt.float16
    Act = mybir.ActivationFunctionType
    Alu = mybir.AluOpType

    nc = bass.Bass(trn_type="TRN2", target_bir_lowering=False, debug=False)
    xt_d = nc.dram_tensor("xt", [D, N], f16, kind="ExternalInput").ap()
    outA_d = nc.dram_tensor("outA", [BS, 9 * BS], f16, kind="ExternalOutput").ap()
    outB_d = nc.dram_tensor("outB", [BS, 8 * BS], f16, kind="ExternalOutput").ap()

    with tile.TileContext(nc) as tc:
        with (
            tc.tile_pool(name="xt", bufs=1) as xt_pool,
            tc.tile_pool(name="sq", bufs=4) as sq_pool,
            tc.tile_pool(name="tmp", bufs=10) as tmp_pool,
            tc.tile_pool(name="nrm", bufs=9) as nrm_pool,
            tc.tile_pool(name="rows", bufs=4) as row_pool,
            tc.tile_pool(name="bc16", bufs=5) as bc_pool,
            tc.tile_pool(name="consts", bufs=1) as const_pool,
            tc.tile_pool(name="stage", bufs=3) as stage_pool,
            tc.tile_pool(name="ps_nb", bufs=2, space=bass.MemorySpace.PSUM) as ps_nb,
            tc.tile_pool(name="ps_g", bufs=6, space=bass.MemorySpace.PSUM) as ps_g,
        ):
            ones_col = const_pool.tile([128, 1], f16, name="ones_col")
            nc.vector.memset(ones_col[:], 1.0)
            one_bias = const_pool.tile([128, 1], f32, name="one_bias")
            nc.vector.memset(one_bias[:], 1.0)

            # xt is tile-major: 512-col tile t occupies [2048t, 2048(t+1)),
            # with k-chunk k at +512k. Every op below then works on plain
            # contiguous 2-dim slices (exact ranges for dep tracking), and
            # the per-tile square/scale run as ONE wide fp16 2x-mode DVE op.
            TW = KC * BS                      # 2048 cols per tile
            xt = xt_pool.tile([128, NB * TW], f16, name="xt")

            def xtile(t):
                return xt[:, TW * t:TW * (t + 1)]

            def xmov(k, t):
                return xt[:, TW * t + BS * k:TW * t + BS * (k + 1)]

            def xw(k, wt, mt):
                base = TW * wt + BS * k + 128 * mt
                return xt[:, base:base + 128]

            # tile-major input DMA, norm-group order
            for tiles in NORM_GROUPS:
                for t in tiles:
                    for k in range(KC):
                        nc.sync.dma_start(
                            out=xmov(k, t),
                            in_=xt_d[k * 128:(k + 1) * 128,
                                     t * BS:(t + 1) * BS])

            ones_row = const_pool.tile([1, 128], f16, name="ones_row")
            nc.vector.memset(ones_row[:], 1.0)

            def normalize(tiles, gid):
                """Column-normalize the 512-wide tiles in place (fp16).
                Per tile: one fused DVE square over the contiguous
                [128, 2048] tile -> 2-level DVE adds -> ones-matmul
                partition reduce -> rsqrt as exp(-0.5*ln(x)) on ACT (the
                [1,512] row lives on one partition, where DVE reciprocal
                costs ~6.3 ns/elem; ACT Rsqrt is blocked for accuracy) ->
                K=1 broadcast matmul -> ACT f16 copy -> one fused in-place
                fp16 2x-mode DVE scale. No DMA in the chain: measured,
                DRAM-bounce variants convoy the DMA queues and lose."""
                for i, t in enumerate(tiles):
                    sq = sq_pool.tile([128, TW], f16, tag="sq", name=f"sq{t}")
                    nc.vector.tensor_mul(sq[:], xtile(t), xtile(t))
                    a01 = tmp_pool.tile([128, 2 * BS], f16, tag="tmp",
                                        name=f"a01_{t}")
                    nc.vector.tensor_add(a01[:], sq[:, 0:2 * BS],
                                         sq[:, 2 * BS:4 * BS])
                    ssum = tmp_pool.tile([128, BS], f16, tag="tmp",
                                         name=f"ssum_{t}")
                    nc.vector.tensor_add(ssum[:], a01[:, 0:BS], a01[:, BS:])
                    n2 = ps_nb.tile([1, BS], f32, tag="nb", name=f"n2_{t}")
                    nc.tensor.matmul(n2[:], ones_col[:], ssum[:],
                                     start=True, stop=True)
                    lnx = tmp_pool.tile([1, BS], f32, tag="tmp",
                                        name=f"ln_{t}")
                    nc.scalar.activation(lnx[:], n2[:], Act.Ln)
                    rn = row_pool.tile([1, BS], f16, tag="rn",
                                       name=f"rn_{gid}_{i}")
                    nc.scalar.activation(rn[:], lnx[:], Act.Exp, scale=-0.5)
                    bc = ps_nb.tile([128, BS], f32, tag="nb", name=f"bc_{t}")
                    nc.tensor.matmul(bc[:], ones_row[:], rn[:],
                                     start=True, stop=True)
                    bc16 = bc_pool.tile([128, BS], f16, tag="bc",
                                        name=f"bc16_{t}")
                    nc.scalar.copy(bc16[:], bc[:])
                    nc.vector.tensor_mul(
                        xtile(t).rearrange("p (a c) -> p a c", a=KC),
                        xtile(t).rearrange("p (a c) -> p a c", a=KC),
                        bc16[:].unsqueeze(1).broadcast_to((128, KC, BS)))

            normalize(NORM_GROUPS[0], 0)

            for gi, (wt, tiles, out_name, off) in enumerate(GRAM_GROUPS):
                out_d = outA_d if out_name == "outA" else outB_d
                nt = len(tiles)
                for mt in range(MT):
                    # prefetch the next norm group under this group's gram
                    # stream; high_priority floats it as early as deps allow
                    if mt == 1 and gi + 1 < len(NORM_GROUPS):
                        with tc.high_priority():
                            normalize(NORM_GROUPS[gi + 1], gi + 1)
                    psums = [ps_g.tile([128, BS], f32, tag="pg",
                                       name=f"pg_{gi}_{mt}_{j}")
                             for j in range(nt)]
                    for k in range(KC):
                        for j, t in enumerate(tiles):
                            nc.tensor.matmul(psums[j][:], xw(k, wt, mt),
                                             xmov(k, t),
                                             start=(k == 0), stop=(k == KC - 1))
                    stage = stage_pool.tile([128, nt * BS], f16, tag="st",
                                            name=f"st_{gi}_{mt}")
                    for j in range(nt):
                        ssl = slice(j * BS, (j + 1) * BS)
                        # ~1/4 of epilogue converts on DVE, rest on ACT
                        if (gi * MT + mt + j) % 4 == 0:
                            nc.vector.tensor_scalar(stage[:, ssl], psums[j][:],
                                                    -1.0, 1.0,
                                                    Alu.mult, Alu.add)
                        else:
                            nc.scalar.activation(stage[:, ssl], psums[j][:],
                                                 Act.Identity,
                                                 bias=one_bias[:], scale=-1.0)
                    nc.sync.dma_start(
                        out=out_d[mt * 128:(mt + 1) * 128, off:off + nt * BS],
                        in_=stage[:])

    _cached[key] = nc
    return nc


def kernel(mapping: np.ndarray) -> np.ndarray:
    from concourse.bass_utils import run_bass_kernel_spmd

    mapping = np.ascontiguousarray(mapping, dtype=np.float32)
    assert mapping.shape == (N, D)
    xt16 = np.ascontiguousarray(mapping.T.astype(np.float16))  # [512, 8192]
    in_maps = []
    for c in range(N_CORES):
        in_maps.append({"xt": np.ascontiguousarray(
            np.roll(xt16, -BS * c, axis=1))})

    nc = _build()

    trace = bool(int(os.environ.get("BASSKNN_TRACE", "0")))
    if trace:
        _install_ntff_hook()
    res = run_bass_kernel_spmd(nc, in_maps, list(range(N_CORES)), trace=trace)
    global LAST_EXEC_NS
    if trace:
        LAST_EXEC_NS = res.exec_time_ns

    full = np.empty((N, N), np.float32)
    for c in range(N_CORES):
        A = np.asarray(res.results[c]["outA"]).astype(np.float32)
        B = np.asarray(res.results[c]["outB"]).astype(np.float32)
        for t in range(9):
            j = (c + t) % NB
            blk = A[:, t * BS:(t + 1) * BS]
            full[c * BS:(c + 1) * BS, j * BS:(j + 1) * BS] = blk
            if t:
                full[j * BS:(j + 1) * BS, c * BS:(c + 1) * BS] = blk.T
        i2 = c + 8
        for e in range(8):
            j = (i2 + e) % NB
            blk = B[:, e * BS:(e + 1) * BS]
            full[i2 * BS:(i2 + 1) * BS, j * BS:(j + 1) * BS] = blk
            if e:
                full[j * BS:(j + 1) * BS, i2 * BS:(i2 + 1) * BS] = blk.T
    return full
